# revision 6
# baseline (speedup 1.0000x reference)
"""Multi-head attention (B=4, S=2048, D=1024, H=16, DK=64) on 8 TRN2 cores.

Sharding: core c = (b, g) with b = c//2 in 0..3 (data parallel on batch) and
g = c%2 (tensor parallel on heads: 8 heads / 512 d' columns per group).
Each core computes a partial output projection; the host sums the two
partials per batch (the "all-reduce" of the sharding hint, done host-side)
and adds bo.

Per-core device algorithm (all matmul inputs bf16, fp32 PSUM accumulation):
  QT[d',q] = (Wq_g^T Xq^T + bq 1^T)     via lhsT=Wq tiles, rhs=XqT tiles
  KT[d',k] = same with Wk
  Vn[k,d'] = Xv Wv_g + 1 bv^T           natural layout, plus a ones column
                                        per head -> V_aug [k, 65] per head
  per (q-chunk, head):
    ST[k,q]   = KT_h^T-style scores via lhsT=KT slice, rhs=QT slice
    P = exp(ST/8)                       one ACT pass per 2 k-tiles (PSUM->SBUF)
    AT_aug    = sum_k V_aug^T P         -> [65, q]: rows 0..63 = V^T P,
                                           row 64 = softmax denominators
    r = 1/AT_aug[64]                    DVE reciprocal_approx, broadcast via
                                        DRAM-bounce DMA to [64, q]
    ATn = AT_aug[0:64] * r              -> attention output, transposed
  out[q,e] += sum_h ATn_h^T Wo_h        partial output projection (fp32 out)
"""

import os
import sys
import time
import types

sys.path.insert(0, "/opt/trn_rl_repo")

import numpy as np
import ml_dtypes

# ---------------------------------------------------------------------------
# axon NTFF profile hook (missing from this image's antenv stub); harmless
# when tracing is disabled.
# ---------------------------------------------------------------------------
def _install_axon_hooks():
    import antenv

    if "antenv.axon_hooks" in sys.modules:
        return
    hooks = types.ModuleType("antenv.axon_hooks")
    hooks._hook = None
    hooks.set_axon_ntff_profile_hook = lambda h: setattr(hooks, "_hook", h)
    hooks.get_axon_ntff_profile_hook = lambda: hooks._hook
    sys.modules["antenv.axon_hooks"] = hooks
    antenv.axon_hooks = hooks
    try:
        from trn_agent_boot.trn_boot import _ntff_profile_via_ctypes

        hooks.set_axon_ntff_profile_hook(
            _ntff_profile_via_ctypes("/opt/axon/libaxon_pjrt.so")
        )
    except Exception:
        pass


_install_axon_hooks()

import concourse.bacc as bacc
import concourse.bass as bass
import concourse.tile as tile
from concourse import mybir
from concourse import bass_utils
from concourse.bass_utils import run_bass_kernel_spmd

# The trace path uploads artifacts to a network bucket; keep it local.
bass_utils.upload_artifacts = lambda tmpdir: tmpdir

BF16 = mybir.dt.bfloat16
F32 = mybir.dt.float32

# Problem dims (hardcoded per spec)
B, S, D = 4, 2048, 1024
H, DK = 16, 64
N_CORES = 8
HC = H // N_CORES * B  # heads per core = 8  (16 heads / 2 groups)
DPC = HC * DK  # d' columns per core = 512

LAST_EXEC_TIME_NS = None


def build_program(s=S, dm=D, hc=HC, e=D):
    """Build the per-core Bass program. All dims in units of elements.

    s: sequence length (multiple of 512), dm: model dim (multiple of 128),
    hc: heads per core (even), e: output model dim (multiple of 512).
    """
    dk = DK
    dpc = hc * dk  # d' per core
    pairs = hc // 2
    dt_n = dm // 128  # d-tiles (contraction tiles for projections)
    st_n = s // 128  # s-tiles = k-tiles
    qc_n = s // 512  # q-chunks
    ec_n = e // 512  # out-proj column chunks
    KG = 2  # k-tiles per exp group
    kg_n = st_n // KG

    nc = bacc.Bacc("TRN2", target_bir_lowering=False, debug=False,
                   num_devices=N_CORES)

    xqT = nc.dram_tensor("xqT", [dm, s], BF16, kind="ExternalInput")
    xkT = nc.dram_tensor("xkT", [dm, s], BF16, kind="ExternalInput")
    xvT = nc.dram_tensor("xvT", [dm, s], BF16, kind="ExternalInput")
    wq = nc.dram_tensor("wq", [dm, dpc], BF16, kind="ExternalInput")
    wk = nc.dram_tensor("wk", [dm, dpc], BF16, kind="ExternalInput")
    wv = nc.dram_tensor("wv", [dm, dpc], BF16, kind="ExternalInput")
    wo = nc.dram_tensor("wo", [dpc, e], BF16, kind="ExternalInput")
    bq = nc.dram_tensor("bq", [dpc], BF16, kind="ExternalInput")
    bk = nc.dram_tensor("bk", [dpc], BF16, kind="ExternalInput")
    bv = nc.dram_tensor("bv", [dpc], BF16, kind="ExternalInput")
    out = nc.dram_tensor("out", [s, e], F32, kind="ExternalOutput")

    with tile.TileContext(nc) as tc:
        with (
            tc.tile_pool(name="singles", bufs=1) as singles,
            tc.tile_pool(name="xin", bufs=2) as xin,
            tc.tile_pool(name="expst", bufs=3) as expst_pool,
            tc.tile_pool(name="atn", bufs=2 * hc) as atn_pool,
            tc.tile_pool(name="small", bufs=4) as small,
            tc.tile_pool(name="outsb", bufs=3) as outsb_pool,
            tc.tile_pool(name="ps512", bufs=2, space="PSUM") as ps512,
            tc.tile_pool(name="ps_sc", bufs=2, space="PSUM") as ps_sc,
            tc.tile_pool(name="ps_at", bufs=2, space="PSUM") as ps_at,
            tc.tile_pool(name="dramb", bufs=4, space="DRAM") as dramb,
        ):
            # ---- persistent SBUF tensors ----
            qt_sb = singles.tile([128, pairs, s], BF16, tag="qt")
            kt_sb = singles.tile([128, pairs, s], BF16, tag="kt")
            vn_sb = singles.tile([128, st_n, hc, dk + 1], BF16, tag="vn")
            wq_sb = singles.tile([128, dt_n, dpc], BF16, tag="wq")
            wk_sb = singles.tile([128, dt_n, dpc], BF16, tag="wk")
            wv_sb = singles.tile([128, dt_n, dpc], BF16, tag="wv")
            wo_sb = singles.tile([64, hc, e], BF16, tag="wo")
            bq_sb = singles.tile([1, dpc], BF16, tag="bq")
            bk_sb = singles.tile([1, dpc], BF16, tag="bk")
            bv_sb = singles.tile([1, dpc], BF16, tag="bv")
            ones_sb = singles.tile([128, 512], BF16, tag="ones")

            nc.sync.dma_start(
                out=wq_sb, in_=wq.ap().rearrange("(t p) n -> p t n", p=128))
            nc.sync.dma_start(
                out=wk_sb, in_=wk.ap().rearrange("(t p) n -> p t n", p=128))
            nc.sync.dma_start(
                out=wv_sb, in_=wv.ap().rearrange("(t p) n -> p t n", p=128))
            nc.sync.dma_start(
                out=wo_sb, in_=wo.ap().rearrange("(h p) e -> p h e", p=64))
            nc.sync.dma_start(out=bq_sb, in_=bq.ap().rearrange("(o n) -> o n", o=1))
            nc.sync.dma_start(out=bk_sb, in_=bk.ap().rearrange("(o n) -> o n", o=1))
            nc.sync.dma_start(out=bv_sb, in_=bv.ap().rearrange("(o n) -> o n", o=1))
            nc.vector.memset(ones_sb, 1.0)
            # ones column of every V_aug head block
            nc.vector.memset(vn_sb[:, :, :, dk : dk + 1], 1.0)

            # ---- stage A: projections ----
            # Q/K in transposed layout: psum[d' 128, q 512] = sum_t W[t]^T X^T[t]
            for xdram, w_sb, b_sb, dst in (
                (xqT, wq_sb, bq_sb, qt_sb),
                (xkT, wk_sb, bk_sb, kt_sb),
            ):
                x_sb = xin.tile([128, dt_n, s], BF16, tag="x")
                nc.sync.dma_start(
                    out=x_sb, in_=xdram.ap().rearrange("(t p) n -> p t n", p=128))
                for p in range(pairs):
                    for qc in range(qc_n):
                        ps = ps512.tile([128, 512], F32, tag="ps512")
                        for t in range(dt_n):
                            nc.tensor.matmul(
                                ps,
                                w_sb[:, t, p * 128 : (p + 1) * 128],
                                x_sb[:, t, qc * 512 : (qc + 1) * 512],
                                start=(t == 0),
                                stop=False,
                            )
                        nc.tensor.matmul(
                            ps,
                            b_sb[0:1, p * 128 : (p + 1) * 128],
                            ones_sb[0:1, 0:512],
                            start=False,
                            stop=True,
                        )
                        nc.vector.tensor_copy(
                            dst[:, p, qc * 512 : (qc + 1) * 512], ps)

            # V natural: psum[k 128, d'] = sum_t XvT[t]^T Wv[t] + 1 bv^T
            xv_sb = xin.tile([128, dt_n, s], BF16, tag="x")
            nc.sync.dma_start(
                out=xv_sb, in_=xvT.ap().rearrange("(t p) n -> p t n", p=128))
            vw = min(512, dpc)
            for st in range(st_n):
                for nchunk in range(dpc // vw):
                    nsl = slice(nchunk * vw, (nchunk + 1) * vw)
                    ps = ps512.tile([128, vw], F32, tag="ps512")
                    for t in range(dt_n):
                        nc.tensor.matmul(
                            ps,
                            xv_sb[:, t, st * 128 : (st + 1) * 128],
                            wv_sb[:, t, nsl],
                            start=(t == 0),
                            stop=False,
                        )
                    nc.tensor.matmul(
                        ps,
                        ones_sb[0:1, 0:128],
                        bv_sb[0:1, nsl],
                        start=False,
                        stop=True,
                    )
                    nc.vector.tensor_copy(
                        vn_sb[
                            :, st,
                            nchunk * (vw // dk) : (nchunk + 1) * (vw // dk),
                            0:dk,
                        ],
                        ps.rearrange("p (h d) -> p h d", d=dk),
                    )

            # ---- stage B: attention per (q-chunk, head) ----
            for qc in range(qc_n):
                qsl = slice(qc * 512, (qc + 1) * 512)
                atn_q = []
                for h in range(hc):
                    p, sub = h // 2, h % 2
                    psl = slice(sub * 64, (sub + 1) * 64)
                    at_ps = ps_at.tile([65, 512], F32, tag="at")
                    for kg in range(kg_n):
                        sc_ps = ps_sc.tile([128, KG * 512], F32, tag="sc")
                        for j in range(KG):
                            kk = kg * KG + j
                            nc.tensor.matmul(
                                sc_ps[:, j * 512 : (j + 1) * 512],
                                kt_sb[psl, p, kk * 128 : (kk + 1) * 128],
                                qt_sb[psl, p, qsl],
                                start=True,
                                stop=True,
                            )
                        exp_sb = expst_pool.tile([128, KG * 512], BF16, tag="e")
                        nc.scalar.activation(
                            exp_sb, sc_ps,
                            mybir.ActivationFunctionType.Exp,
                            scale=1.0 / np.sqrt(dk),
                        )
                        for j in range(KG):
                            kk = kg * KG + j
                            nc.tensor.matmul(
                                at_ps,
                                vn_sb[:, kk, h, :],
                                exp_sb[:, j * 512 : (j + 1) * 512],
                                start=(kk == 0),
                                stop=(kk == st_n - 1),
                            )
                    # normalization: r = 1/rowsum, broadcast to 64 partitions
                    rs_sb = small.tile([65, 512], F32, tag="rs")
                    nc.vector.reciprocal(
                        out=rs_sb[64:65, :], in_=at_ps[64:65, :])
                    rs_dram = dramb.tile([1, 512], F32, tag="rsd")
                    nc.sync.dma_start(out=rs_dram, in_=rs_sb[64:65, :])
                    bc_sb = small.tile([64, 512], F32, tag="bc")
                    bcast_src = bass.AP(
                        tensor=rs_dram.tensor,
                        offset=rs_dram.offset,
                        ap=[[0, 64]] + list(rs_dram.ap[1:]),
                    )
                    nc.sync.dma_start(out=bc_sb, in_=bcast_src)
                    atn = atn_pool.tile([64, 512], BF16, tag="atn")
                    nc.vector.tensor_mul(atn, at_ps[0:64, :], bc_sb)
                    atn_q.append(atn)

                # ---- stage C: partial out-projection for this q-chunk ----
                for qt_i in range(4):
                    q0 = qc * 4 + qt_i
                    for ecc in range(ec_n):
                        esl = slice(ecc * 512, (ecc + 1) * 512)
                        o_ps = ps512.tile([128, 512], F32, tag="ps512")
                        for h in range(hc):
                            nc.tensor.matmul(
                                o_ps,
                                atn_q[h][:, qt_i * 128 : (qt_i + 1) * 128],
                                wo_sb[:, h, esl],
                                start=(h == 0),
                                stop=(h == hc - 1),
                            )
                        o_sb = outsb_pool.tile([128, 512], F32, tag="o")
                        nc.vector.tensor_copy(o_sb, o_ps)
                        nc.sync.dma_start(
                            out=out.ap()[q0 * 128 : (q0 + 1) * 128, esl],
                            in_=o_sb,
                        )

    nc.compile()
    return nc


_PROGRAM_CACHE = {}


def _get_program(key):
    if key not in _PROGRAM_CACHE:
        _PROGRAM_CACHE[key] = build_program(*key)
    return _PROGRAM_CACHE[key]


def kernel(queries, keys, values, Wq, bq, Wk, bk, Wv, bv, Wo, bo):
    global LAST_EXEC_TIME_NS
    bf16 = ml_dtypes.bfloat16

    nc = _get_program((S, D, HC, D))

    xT = {}
    for name, arr in (("q", queries), ("k", keys), ("v", values)):
        xT[name] = [
            np.ascontiguousarray(np.asarray(arr[b]).T).astype(bf16)
            for b in range(B)
        ]
    Wq, Wk, Wv, Wo = (np.asarray(w) for w in (Wq, Wk, Wv, Wo))
    bqv, bkv, bvv = (np.asarray(v) for v in (bq, bk, bv))

    in_maps = []
    for c in range(N_CORES):
        b, g = c // 2, c % 2
        csl = slice(g * DPC, (g + 1) * DPC)
        in_maps.append(
            {
                "xqT": xT["q"][b],
                "xkT": xT["k"][b],
                "xvT": xT["v"][b],
                "wq": np.ascontiguousarray(Wq[:, csl]).astype(bf16),
                "wk": np.ascontiguousarray(Wk[:, csl]).astype(bf16),
                "wv": np.ascontiguousarray(Wv[:, csl]).astype(bf16),
                "wo": np.ascontiguousarray(Wo[csl, :]).astype(bf16),
                "bq": np.ascontiguousarray(bqv[csl]).astype(bf16),
                "bk": np.ascontiguousarray(bkv[csl]).astype(bf16),
                "bv": np.ascontiguousarray(bvv[csl]).astype(bf16),
            }
        )

    trace = os.environ.get("KERNEL_TRACE", "0") == "1"
    res = run_bass_kernel_spmd(nc, in_maps, list(range(N_CORES)), trace=trace)
    LAST_EXEC_TIME_NS = res.exec_time_ns

    bo = np.asarray(bo, dtype=np.float32)
    out = np.empty((B, S, D), dtype=np.float32)
    for b in range(B):
        out[b] = res.results[2 * b]["out"] + res.results[2 * b + 1]["out"] + bo
    return out


if __name__ == "__main__":
    rng = np.random.default_rng(0)
    t0 = time.time()
    nc = _get_program((S, D, HC, D))
    print(f"build+compile: {time.time() - t0:.1f}s")


# revision 10
# speedup vs baseline: 1.1358x; 1.1358x over previous
"""Multi-head attention (B=4, S=2048, D=1024, H=16, DK=64) on 8 TRN2 cores.

Sharding: core c = (b, g) with b = c//2 in 0..3 (data parallel on batch) and
g = c%2 (tensor parallel on heads: 8 heads / 512 d' columns per group).
Each core computes a partial output projection; the host sums the two
partials per batch (the "all-reduce" of the sharding hint, done host-side)
and adds bo.

Per-core device algorithm (all matmul inputs bf16, fp32 PSUM accumulation):
  QT[d',q] = (Wq_g^T Xq^T + bq 1^T)     via lhsT=Wq tiles, rhs=XqT tiles
  KT[d',k] = same with Wk
  Vn[k,d'] = Xv Wv_g + 1 bv^T           natural layout, plus a ones column
                                        per head -> V_aug [k, 65] per head
  per (q-chunk, head):
    ST[k,q]   = KT_h^T-style scores via lhsT=KT slice, rhs=QT slice
    P = exp(ST/8)                       one ACT pass per 2 k-tiles (PSUM->SBUF)
    AT_aug    = sum_k V_aug^T P         -> [65, q]: rows 0..63 = V^T P,
                                           row 64 = softmax denominators
    r = 1/AT_aug[64]                    DVE reciprocal_approx, broadcast via
                                        DRAM-bounce DMA to [64, q]
    ATn = AT_aug[0:64] * r              -> attention output, transposed
  out[q,e] += sum_h ATn_h^T Wo_h        partial output projection (fp32 out)
"""

import os
import sys
import time
import types

sys.path.insert(0, "/opt/trn_rl_repo")

import numpy as np
import ml_dtypes

# ---------------------------------------------------------------------------
# axon NTFF profile hook (missing from this image's antenv stub); harmless
# when tracing is disabled.
# ---------------------------------------------------------------------------
def _install_axon_hooks():
    import antenv

    if "antenv.axon_hooks" in sys.modules:
        return
    hooks = types.ModuleType("antenv.axon_hooks")
    hooks._hook = None
    hooks.set_axon_ntff_profile_hook = lambda h: setattr(hooks, "_hook", h)
    hooks.get_axon_ntff_profile_hook = lambda: hooks._hook
    sys.modules["antenv.axon_hooks"] = hooks
    antenv.axon_hooks = hooks
    try:
        from trn_agent_boot.trn_boot import _ntff_profile_via_ctypes

        hooks.set_axon_ntff_profile_hook(
            _ntff_profile_via_ctypes("/opt/axon/libaxon_pjrt.so")
        )
    except Exception:
        pass


_install_axon_hooks()

import concourse.bacc as bacc
import concourse.bass as bass
import concourse.tile as tile
from concourse import mybir
from concourse import bass_utils
from concourse.bass_utils import run_bass_kernel_spmd

# The trace path uploads artifacts to a network bucket; keep it local.
bass_utils.upload_artifacts = lambda tmpdir: tmpdir

BF16 = mybir.dt.bfloat16
F32 = mybir.dt.float32

# Problem dims (hardcoded per spec)
B, S, D = 4, 2048, 1024
H, DK = 16, 64
N_CORES = 8
HC = H // N_CORES * B  # heads per core = 8  (16 heads / 2 groups)
DPC = HC * DK  # d' columns per core = 512

LAST_EXEC_TIME_NS = None


def build_program(s=S, dm=D, hc=HC, e=D):
    """Build the per-core Bass program. All dims in units of elements.

    s: sequence length (multiple of 512), dm: model dim (multiple of 128),
    hc: heads per core (even), e: output model dim (multiple of 512).
    """
    dk = DK
    dpc = hc * dk  # d' per core
    pairs = hc // 2
    dt_n = dm // 128  # d-tiles (contraction tiles for projections)
    st_n = s // 128  # s-tiles = k-tiles
    qc_n = s // 512  # q-chunks
    ec_n = e // 512  # out-proj column chunks
    KG = 2  # k-tiles per exp group
    kg_n = st_n // KG

    nc = bacc.Bacc("TRN2", target_bir_lowering=False, debug=False,
                   num_devices=N_CORES)

    xqT = nc.dram_tensor("xqT", [dm, s], BF16, kind="ExternalInput")
    xkT = nc.dram_tensor("xkT", [dm, s], BF16, kind="ExternalInput")
    xvT = nc.dram_tensor("xvT", [dm, s], BF16, kind="ExternalInput")
    wq = nc.dram_tensor("wq", [dm, dpc], BF16, kind="ExternalInput")
    wk = nc.dram_tensor("wk", [dm, dpc], BF16, kind="ExternalInput")
    wv = nc.dram_tensor("wv", [dm, dpc], BF16, kind="ExternalInput")
    wo = nc.dram_tensor("wo", [dpc, e], BF16, kind="ExternalInput")
    bq = nc.dram_tensor("bq", [dpc], BF16, kind="ExternalInput")
    bk = nc.dram_tensor("bk", [dpc], BF16, kind="ExternalInput")
    bv = nc.dram_tensor("bv", [dpc], BF16, kind="ExternalInput")
    out = nc.dram_tensor("out", [s, e], F32, kind="ExternalOutput")

    with tile.TileContext(nc) as tc:
        with (
            tc.tile_pool(name="singles", bufs=1) as singles,
            tc.tile_pool(name="xin", bufs=2) as xin,
            tc.tile_pool(name="expst", bufs=3) as expst_pool,
            tc.tile_pool(name="atn", bufs=2 * hc) as atn_pool,
            tc.tile_pool(name="small", bufs=2) as small,
            tc.tile_pool(name="outsb", bufs=3) as outsb_pool,
            tc.tile_pool(name="ps512", bufs=2, space="PSUM") as ps512,
            tc.tile_pool(name="ps_sc", bufs=2, space="PSUM") as ps_sc,
            tc.tile_pool(name="ps_at", bufs=2, space="PSUM") as ps_at,
            tc.tile_pool(name="dramb", bufs=4, space="DRAM") as dramb,
        ):
            # ---- persistent SBUF tensors ----
            qt_sb = singles.tile([128, pairs, s], BF16, tag="qt")
            kt_sb = singles.tile([128, pairs, s], BF16, tag="kt")
            vn_sb = singles.tile([128, st_n, hc, dk + 1], BF16, tag="vn")
            wq_sb = singles.tile([128, dt_n, dpc], BF16, tag="wq")
            wk_sb = singles.tile([128, dt_n, dpc], BF16, tag="wk")
            wv_sb = singles.tile([128, dt_n, dpc], BF16, tag="wv")
            wo_sb = singles.tile([64, hc, e], BF16, tag="wo")
            bq_sb = singles.tile([1, dpc], BF16, tag="bq")
            bk_sb = singles.tile([1, dpc], BF16, tag="bk")
            bv_sb = singles.tile([1, dpc], BF16, tag="bv")
            ones_sb = singles.tile([128, 512], BF16, tag="ones")

            nc.sync.dma_start(
                out=wq_sb, in_=wq.ap().rearrange("(t p) n -> p t n", p=128))
            nc.sync.dma_start(
                out=wk_sb, in_=wk.ap().rearrange("(t p) n -> p t n", p=128))
            nc.sync.dma_start(
                out=wv_sb, in_=wv.ap().rearrange("(t p) n -> p t n", p=128))
            nc.sync.dma_start(
                out=wo_sb, in_=wo.ap().rearrange("(h p) e -> p h e", p=64))
            nc.sync.dma_start(out=bq_sb, in_=bq.ap().rearrange("(o n) -> o n", o=1))
            nc.sync.dma_start(out=bk_sb, in_=bk.ap().rearrange("(o n) -> o n", o=1))
            nc.sync.dma_start(out=bv_sb, in_=bv.ap().rearrange("(o n) -> o n", o=1))
            nc.vector.memset(ones_sb, 1.0)
            # ones column of every V_aug head block
            nc.vector.memset(vn_sb[:, :, :, dk : dk + 1], 1.0)

            # Warm-up exp ACT right away: forces the ~2.7us ACT_TABLE_LOAD to
            # overlap the projection matmuls instead of stalling the PE >3.4us
            # at the start of attention (which re-throttles HAM to 1.2 GHz
            # for the whole attention phase).
            warm_sb = singles.tile([128, 32], F32, tag="warm")
            nc.scalar.activation(
                warm_sb, ones_sb[:, 0:32], mybir.ActivationFunctionType.Exp)

            # ---- stage A: projections ----
            # Q/K in transposed layout: psum[d' 128, q 512] = sum_t W[t]^T X^T[t]
            for xdram, w_sb, b_sb, dst in (
                (xqT, wq_sb, bq_sb, qt_sb),
                (xkT, wk_sb, bk_sb, kt_sb),
            ):
                x_sb = xin.tile([128, dt_n, s], BF16, tag="x")
                nc.sync.dma_start(
                    out=x_sb, in_=xdram.ap().rearrange("(t p) n -> p t n", p=128))
                for p in range(pairs):
                    for qc in range(qc_n):
                        ps = ps512.tile([128, 512], F32, tag="ps512")
                        for t in range(dt_n):
                            nc.tensor.matmul(
                                ps,
                                w_sb[:, t, p * 128 : (p + 1) * 128],
                                x_sb[:, t, qc * 512 : (qc + 1) * 512],
                                start=(t == 0),
                                stop=False,
                            )
                        nc.tensor.matmul(
                            ps,
                            b_sb[0:1, p * 128 : (p + 1) * 128],
                            ones_sb[0:1, 0:512],
                            start=False,
                            stop=True,
                        )
                        nc.vector.tensor_copy(
                            dst[:, p, qc * 512 : (qc + 1) * 512], ps)

            # V natural: psum[k 128, d'] = sum_t XvT[t]^T Wv[t] + 1 bv^T
            xv_sb = xin.tile([128, dt_n, s], BF16, tag="x")
            nc.sync.dma_start(
                out=xv_sb, in_=xvT.ap().rearrange("(t p) n -> p t n", p=128))
            vw = min(512, dpc)
            for st in range(st_n):
                for nchunk in range(dpc // vw):
                    nsl = slice(nchunk * vw, (nchunk + 1) * vw)
                    ps = ps512.tile([128, vw], F32, tag="ps512")
                    for t in range(dt_n):
                        nc.tensor.matmul(
                            ps,
                            xv_sb[:, t, st * 128 : (st + 1) * 128],
                            wv_sb[:, t, nsl],
                            start=(t == 0),
                            stop=False,
                        )
                    nc.tensor.matmul(
                        ps,
                        ones_sb[0:1, 0:128],
                        bv_sb[0:1, nsl],
                        start=False,
                        stop=True,
                    )
                    nc.vector.tensor_copy(
                        vn_sb[
                            :, st,
                            nchunk * (vw // dk) : (nchunk + 1) * (vw // dk),
                            0:dk,
                        ],
                        ps.rearrange("p (h d) -> p h d", d=dk),
                    )

            # ---- stage B: attention per (q-chunk, head) ----
            for qc in range(qc_n):
                qsl = slice(qc * 512, (qc + 1) * 512)
                atn_q = []
                rs_dram = dramb.tile([hc, 512], F32, tag="rsd")
                for h in range(hc):
                    p, sub = h // 2, h % 2
                    psl = slice(sub * 64, (sub + 1) * 64)
                    at_ps = ps_at.tile([65, 512], F32, tag="at")
                    for kg in range(kg_n):
                        sc_ps = ps_sc.tile([128, KG * 512], F32, tag="sc")
                        for j in range(KG):
                            kk = kg * KG + j
                            nc.tensor.matmul(
                                sc_ps[:, j * 512 : (j + 1) * 512],
                                kt_sb[psl, p, kk * 128 : (kk + 1) * 128],
                                qt_sb[psl, p, qsl],
                                start=True,
                                stop=True,
                            )
                        exp_sb = expst_pool.tile([128, KG * 512], BF16, tag="e")
                        nc.scalar.activation(
                            exp_sb, sc_ps,
                            mybir.ActivationFunctionType.Exp,
                            scale=1.0 / np.sqrt(dk),
                        )
                        for j in range(KG):
                            kk = kg * KG + j
                            nc.tensor.matmul(
                                at_ps,
                                vn_sb[:, kk, h, :],
                                exp_sb[:, j * 512 : (j + 1) * 512],
                                start=(kk == 0),
                                stop=(kk == st_n - 1),
                            )
                    # stash unnormalized head output + its denominator row;
                    # the reciprocal is batched over all heads of the q-chunk
                    atn = atn_pool.tile([64, 512], BF16, tag="atn")
                    nc.vector.tensor_copy(atn, at_ps[0:64, :])
                    rs_row = small.tile([65, 512], F32, tag="rsrow")
                    nc.vector.tensor_copy(rs_row[64:65, :], at_ps[64:65, :])
                    nc.sync.dma_start(
                        out=rs_dram[h : h + 1, :], in_=rs_row[64:65, :])
                    atn_q.append(atn)

                # batched normalization for the whole q-chunk
                rs_sb = small.tile([hc, 512], F32, tag="rs")
                nc.sync.dma_start(out=rs_sb, in_=rs_dram)
                rec_sb = small.tile([hc, 512], F32, tag="rec")
                nc.vector.reciprocal(out=rec_sb, in_=rs_sb)
                rec_dram = dramb.tile([hc, 512], F32, tag="recd")
                nc.sync.dma_start(out=rec_dram, in_=rec_sb)
                for h in range(hc):
                    row = rec_dram[h : h + 1, :]
                    bc_sb = small.tile([64, 512], F32, tag="bc")
                    bcast_src = bass.AP(
                        tensor=row.tensor,
                        offset=row.offset,
                        ap=[[0, 64]] + list(row.ap[1:]),
                    )
                    nc.sync.dma_start(out=bc_sb, in_=bcast_src)
                    nc.vector.tensor_mul(atn_q[h], atn_q[h], bc_sb)

                # ---- stage C: partial out-projection for this q-chunk ----
                for qt_i in range(4):
                    q0 = qc * 4 + qt_i
                    for ecc in range(ec_n):
                        esl = slice(ecc * 512, (ecc + 1) * 512)
                        o_ps = ps512.tile([128, 512], F32, tag="ps512")
                        for h in range(hc):
                            nc.tensor.matmul(
                                o_ps,
                                atn_q[h][:, qt_i * 128 : (qt_i + 1) * 128],
                                wo_sb[:, h, esl],
                                start=(h == 0),
                                stop=(h == hc - 1),
                            )
                        o_sb = outsb_pool.tile([128, 512], F32, tag="o")
                        nc.vector.tensor_copy(o_sb, o_ps)
                        nc.sync.dma_start(
                            out=out.ap()[q0 * 128 : (q0 + 1) * 128, esl],
                            in_=o_sb,
                        )

    nc.compile()
    return nc


_PROGRAM_CACHE = {}


def _get_program(key):
    if key not in _PROGRAM_CACHE:
        _PROGRAM_CACHE[key] = build_program(*key)
    return _PROGRAM_CACHE[key]


def kernel(queries, keys, values, Wq, bq, Wk, bk, Wv, bv, Wo, bo):
    global LAST_EXEC_TIME_NS
    bf16 = ml_dtypes.bfloat16

    nc = _get_program((S, D, HC, D))

    xT = {}
    for name, arr in (("q", queries), ("k", keys), ("v", values)):
        xT[name] = [
            np.ascontiguousarray(np.asarray(arr[b]).T).astype(bf16)
            for b in range(B)
        ]
    Wq, Wk, Wv, Wo = (np.asarray(w) for w in (Wq, Wk, Wv, Wo))
    bqv, bkv, bvv = (np.asarray(v) for v in (bq, bk, bv))

    in_maps = []
    for c in range(N_CORES):
        b, g = c // 2, c % 2
        csl = slice(g * DPC, (g + 1) * DPC)
        in_maps.append(
            {
                "xqT": xT["q"][b],
                "xkT": xT["k"][b],
                "xvT": xT["v"][b],
                "wq": np.ascontiguousarray(Wq[:, csl]).astype(bf16),
                "wk": np.ascontiguousarray(Wk[:, csl]).astype(bf16),
                "wv": np.ascontiguousarray(Wv[:, csl]).astype(bf16),
                "wo": np.ascontiguousarray(Wo[csl, :]).astype(bf16),
                "bq": np.ascontiguousarray(bqv[csl]).astype(bf16),
                "bk": np.ascontiguousarray(bkv[csl]).astype(bf16),
                "bv": np.ascontiguousarray(bvv[csl]).astype(bf16),
            }
        )

    trace = os.environ.get("KERNEL_TRACE", "0") == "1"
    res = run_bass_kernel_spmd(nc, in_maps, list(range(N_CORES)), trace=trace)
    LAST_EXEC_TIME_NS = res.exec_time_ns

    bo = np.asarray(bo, dtype=np.float32)
    out = np.empty((B, S, D), dtype=np.float32)
    for b in range(B):
        out[b] = res.results[2 * b]["out"] + res.results[2 * b + 1]["out"] + bo
    return out


if __name__ == "__main__":
    rng = np.random.default_rng(0)
    t0 = time.time()
    nc = _get_program((S, D, HC, D))
    print(f"build+compile: {time.time() - t0:.1f}s")
